# revision 49
# baseline (speedup 1.0000x reference)
"""Trainium2 Bass kernel for the MAB (multihead attention block) problem.

Full inputs in, full outputs out. Data-parallel over batch: 16 batches
across 8 NeuronCores = 2 batches/core. No collectives.

Vs the bf16 baseline:
  * PV matmuls run in fp8e4 with MatmulPerfMode.DoubleRow: one matmul
    consumes TWO nk-chunks (planes along the nk-tile axis) at the same
    per-instruction cost as one bf16 chunk (HW-verified 217ns vs 216ns)
    -> 2x on the PV half of attention. V is scaled by 4 (fp8 range) with
    a 4.0-ones column so the softmax denominator cancels the factor.
  * Scores stay bf16 (64-row j-alternating matmuls hide LDWEIGHTS;
    DoubleRow's 256-column weight load would double their cost).
  * exp writes fp8e4 P directly; a tunable subset of exp tiles runs on
    the DVE as a fast-exp (round(x*c1+c2) into a uint8 view of the fp8
    buffer; softmax normalization cancels the approximation bias).
  * ACT runs only Exp-set functions (Exp/Copy) - LN rsqrt is a bit-hack
    + 2 Newton steps on the DVE, so no activation-table swaps.
  * FFN relu+residual fused into one DVE scalar_tensor_tensor; LN2
    stats batched so the rsqrt chain runs once per batch.
  * Projections stream 1024 columns per matmul (2KB moving-operand cap).
"""

import math
import sys
from contextlib import ExitStack

import numpy as np

sys.path.insert(0, "/opt/trn_rl_repo")

import concourse.bass as bass
import concourse.tile as tile
from concourse import bacc
from concourse import mybir
from concourse.bass import ds, ts
from concourse.bass_utils import run_bass_kernel_spmd
from concourse.masks import make_identity

FP = mybir.dt.float32
BF = mybir.dt.bfloat16
F8 = mybir.dt.float8e4
U8 = mybir.dt.uint8
I32 = mybir.dt.int32
AF = mybir.ActivationFunctionType
ALU = mybir.AluOpType
DR = mybir.MatmulPerfMode.DoubleRow

B, N, D = 16, 1024, 512
NCORES = 8
BL = B // NCORES  # batches per core
H, HD = 8, 64
PAIRS = H // 2
SCALE = 1.0 / math.sqrt(D)
EPS = 1e-5
P = 128
DT = D // P  # 4 dv chunks
NT = N // P  # 8 nq/nk tiles
HA = HD + 1  # head dim + denominator column
HB = HA + 1  # 66: pad so bf16 PSUM blocks stay 4B-aligned
LOG2E = 1.4426950408889634
# fast-exp constants: fp8e4 bits of exp(x*SCALE) ~= round(x*FE_C1 + FE_C2)
FE_C1 = 8.0 * LOG2E * SCALE
FE_C2 = 55.6
# which (group, m) exp tiles run on DVE as fast-exp (group = hp*2+hf)
FASTEXP_DVE = {(gi, 5) for gi in range(8)}


def _bcast_ap(ap):
    """Broadcast a [D]-shaped DRAM AP across all 128 partitions."""
    return bass.AP(tensor=ap.tensor, offset=ap.offset, ap=[[0, P]] + list(ap.ap))


def _free_bcast(tileap, inner):
    """[P, K] tile viewed as [P, K, inner] with stride-0 inner dim."""
    return bass.AP(
        tensor=tileap.tensor,
        offset=tileap.offset,
        ap=[list(tileap.ap[0]), list(tileap.ap[1]), [0, inner]],
    )


def _build_program(triv0, triv1, trivbo):
    nc = bacc.Bacc(None, target_bir_lowering=False)
    dr = {}
    for name, shape in [
        ("QT", [BL, D, N]),
        ("KT", [BL, D, N]),  # fp8e4
        ("Wq", [D, D]),
        ("Wk", [D, D]),  # fp8e4
        ("Wv", [D, D]),
        ("Wo", [D, D]),
        ("bq2", [P, DT]),
        ("bk2", [P, DT]),
        ("bv4", [D]),  # 4 * bv
        ("bo", [D]),
        ("g0", [D]),
        ("b0", [D]),
        ("g1", [D]),
        ("b1", [D]),
    ]:
        if name in ("KT", "Wk"):
            dt = F8
        elif name in ("QT", "Wq", "Wv", "Wo"):
            dt = BF
        else:
            dt = FP
        dr[name] = nc.declare_dram_parameter(name, shape, dt, isOutput=False)
    out_O = nc.declare_dram_parameter("O", [BL, N, D], FP, isOutput=True)

    qt_src = dr["QT"][:].rearrange("b (c p) n -> b p c n", p=P)
    kt_src = dr["KT"][:].rearrange("b (c p) n -> b p c n", p=P)

    with tile.TileContext(nc) as tc, ExitStack() as ctx:
        singles = ctx.enter_context(tc.tile_pool(name="singles", bufs=1))
        work = ctx.enter_context(tc.tile_pool(name="work", bufs=1))
        abuf = ctx.enter_context(tc.tile_pool(name="abuf", bufs=2))
        pch = ctx.enter_context(tc.tile_pool(name="pch", bufs=5))
        lnt = ctx.enter_context(tc.tile_pool(name="lnt", bufs=2))
        ost = ctx.enter_context(tc.tile_pool(name="ost", bufs=2))
        otile = ctx.enter_context(tc.tile_pool(name="otile", bufs=2))
        sml = ctx.enter_context(tc.tile_pool(name="sml", bufs=8))
        # PSUM: flow 2x2 banks (score tiles) + opv 2 + acc 2x1 = 8 banks
        ps_acc = ctx.enter_context(tc.tile_pool(name="ps_acc", bufs=2, space="PSUM"))
        ps_pv = ctx.enter_context(tc.tile_pool(name="ps_pv", bufs=1, space="PSUM"))
        ps_flow = ctx.enter_context(tc.tile_pool(name="ps_flow", bufs=2, space="PSUM"))

        # ---- statics. Wq rides the (otherwise idle) gpsimd queue so the
        # sync queue can stream the qt chunks immediately; the slow
        # partition-broadcast DMAs go after it.
        wsb = {}
        for wname in ("Wq", "Wk", "Wv", "Wo"):
            wdt = F8 if wname == "Wk" else BF
            wsb[wname] = singles.tile([P, DT, D], wdt, tag=wname, name=wname)
        nc.gpsimd.dma_start(
            out=wsb["Wq"], in_=dr["Wq"][:].rearrange("(c p) d -> p c d", p=P)
        )
        bq_sb = singles.tile([P, DT], FP, tag="bq2")
        nc.sync.dma_start(out=bq_sb, in_=dr["bq2"][:])
        bk_sb = singles.tile([P, DT], FP, tag="bk2")
        nc.sync.dma_start(out=bk_sb, in_=dr["bk2"][:])
        bc = {}
        for bname in ("bv4", "bo", "g0", "b0", "g1", "b1"):
            t = singles.tile([P, D], FP, tag=bname)
            nc.gpsimd.dma_start(out=t, in_=_bcast_ap(dr[bname][:]))
            bc[bname] = t
        ident = singles.tile([P, P], FP, tag="ident")
        make_identity(nc, ident)
        ident_b = singles.tile([P, P], BF, tag="identb")
        nc.vector.tensor_copy(ident_b, ident)

        state = {}

        def rsqrt_dve(out_ap, in_ap, n):
            """out = 1/sqrt(in) via bit hack + 2 Newton iters. [P, n] fp32."""
            yh = sml.tile([P, n], FP, tag=f"rs_a{n}", name="rs_a")
            t0 = sml.tile([P, n], FP, tag=f"rs_b{n}", name="rs_b")
            nc.vector.tensor_scalar(
                out=yh[:].bitcast(I32), in0=in_ap.bitcast(I32),
                scalar1=1, scalar2=None, op0=ALU.logical_shift_right,
            )
            nc.vector.tensor_scalar(
                out=out_ap.bitcast(I32), in0=yh[:].bitcast(I32),
                scalar1=-1, scalar2=0x5F3759DF, op0=ALU.mult, op1=ALU.add,
            )
            for _ in range(2):
                nc.vector.tensor_tensor(t0, in_ap, out_ap, ALU.mult)
                nc.vector.tensor_tensor(t0, t0, out_ap, ALU.mult)
                nc.vector.tensor_scalar(
                    out=t0, in0=t0, scalar1=-0.5, scalar2=1.5,
                    op0=ALU.mult, op1=ALU.add,
                )
                nc.vector.tensor_tensor(out_ap, out_ap, t0, ALU.mult)

        def phase_a_load(b):
            """DMA inputs into the (double-buffered) A tiles."""
            st = {}
            qt = abuf.tile([P, DT, N], BF, tag="qt")
            k8 = abuf.tile([P, DT, N], F8, tag="k8")
            for c in range(DT):
                nc.sync.dma_start(out=qt[:, c, :], in_=qt_src[b, :, c, :])
                nc.scalar.dma_start(out=k8[:, c, :], in_=kt_src[b, :, c, :])
            if b == 0:
                # Wk/Wv ride the ACT queue behind the k8 chunks; Wo (needed
                # only in phase C) goes to sync after the inputs.
                for wname, eng in (("Wk", nc.scalar), ("Wv", nc.scalar),
                                   ("Wo", nc.sync)):
                    eng.dma_start(
                        out=wsb[wname],
                        in_=dr[wname][:].rearrange("(c p) d -> p c d", p=P),
                    )
            st.update(qt=qt, k8=k8)
            state[b] = st

        def phase_a(b, merge):
            """Emit projections now (merge=False) or hand the per-chunk
            emitters to phase_b (merge=True) so batch-0 compute starts as
            soon as chunk 0 lands."""
            st = state[b]
            qt, k8 = st["qt"], st["k8"]

            qpt = abuf.tile([P, DT, N], BF, tag="qpt")
            kpt = abuf.tile([P, DT, N], BF, tag="kpt")

            def proj_chunk(t):
                ps = ps_flow.tile([P, N], FP, tag="flow", name="projps")
                for hf in range(2):
                    for c in range(DT):
                        nc.tensor.matmul(
                            ps[:, ds(hf * 512, 512)],
                            wsb["Wq"][:, c, ts(t, P)],
                            qt[:, c, ds(hf * 512, 512)],
                            start=(c == 0),
                            stop=(c == DT - 1),
                        )
                nc.vector.tensor_scalar_add(qpt[:, t, :], ps, bq_sb[:, t : t + 1])
                ps = ps_flow.tile([P, N], FP, tag="flow", name="projps")
                for hf in range(2):
                    for cp in range(2):
                        # fp8 DoubleRow: one matmul covers 2 contraction chunks
                        nc.tensor.matmul(
                            ps[:, ds(hf * 512, 512)],
                            wsb["Wk"][:, ds(2 * cp, 2), ts(t, P)],
                            k8[:, ds(2 * cp, 2), ds(hf * 512, 512)],
                            start=(cp == 0),
                            stop=(cp == 1),
                            perf_mode=DR,
                        )
                nc.vector.tensor_scalar_add(kpt[:, t, :], ps, bk_sb[:, t : t + 1])

            # Vp natural, fp8, augmented: per head 4*V (64 cols) + a 4.0 column
            vpa = abuf.tile([P, NT, H * HB], F8, tag="vpa")
            ones_ap = bass.AP(
                tensor=vpa.tensor, offset=vpa.offset + HD,
                ap=[list(vpa.ap[0]), [H * HB, NT], [HB, H], [1, 1]],
            )
            nc.vector.memset(ones_ap, 4.0)

            def emit_vp(m):
                ps = ps_acc.tile([P, 512], FP, tag="acc", name="vps")
                for c in range(DT):
                    nc.tensor.matmul(
                        ps,
                        k8[:, c, ts(m, P)],
                        wsb["Wv"][:, c, :],
                        start=(c == 0),
                        stop=(c == DT - 1),
                    )
                vslice = bass.AP(
                    tensor=vpa.tensor, offset=vpa.offset + m * (H * HB),
                    ap=[list(vpa.ap[0]), [HB, H], [1, HD]],
                )
                # vpa = 4*Vp + 4*bv  (bv4 = 4*bv from host)
                nc.vector.scalar_tensor_tensor(
                    out=vslice,
                    in0=ps[:, :].rearrange("p (h s) -> p h s", s=HD),
                    scalar=4.0,
                    in1=bc["bv4"][:, :].rearrange("p (h s) -> p h s", s=HD),
                    op0=ALU.mult,
                    op1=ALU.add,
                )

            qp = abuf.tile([P, NT, D], BF, tag="qp")

            if merge:
                st_proj = [lambda t=t: proj_chunk(t) for t in range(DT)]
                st_vp_rest = [lambda m=m: emit_vp(m) for m in range(NT)]
            else:
                for t in range(DT):
                    proj_chunk(t)
                for m in range(4):
                    emit_vp(m)
                st_proj = []
                st_vp_rest = [lambda m=m: emit_vp(m) for m in range(4, NT)]

            st.update(qpt=qpt, kpt=kpt, vpa=vpa, qp=qp,
                      vp_rest=st_vp_rest, proj=st_proj)

        def qp_chunk(b, t):
            st = state[b]
            qpt, qp = st["qpt"], st["qp"]
            for half in range(2):
                tp = ps_acc.tile([P, 512], BF, tag="acc", name="qptr")
                for mm in range(4):
                    m = half * 4 + mm
                    nc.tensor.transpose(
                        tp[:, ts(mm, P)], qpt[:, t, ts(m, P)], ident_b
                    )
                nc.any.tensor_copy(
                    qp[:, ds(half * 4, 4), ts(t, P)],
                    tp.rearrange("p (mm n) -> p mm n", n=P),
                )

        def phase_b(b, order, extras=None):
            st = state[b]
            gi = [0]
            qpt, kpt, vpa, qp = st["qpt"], st["kpt"], st["vpa"], st["qp"]
            oasm = abuf.tile([P, NT, D], BF, tag="oasm")
            st["oasm"] = oasm
            pending_drain = [None]
            carry = []  # deferred PV matmul thunks crossing group boundary
            proj_done = set()

            for hp, hf in order:
                if True:
                    if hp not in proj_done:
                        proj_done.add(hp)
                        if st["proj"]:
                            st["proj"].pop(0)()
                    qslice = ds(hf * 512, 512)
                    o_pair = ps_pv.tile([HA, N], FP, tag="opv")

                    def emit_pv(t, p2, o_pair=o_pair, hp=hp):
                        for j in range(2):
                            lhs = bass.AP(
                                tensor=vpa.tensor,
                                offset=vpa.offset
                                + 2 * t * (H * HB)
                                + (2 * hp + j) * HB,
                                ap=[list(vpa.ap[0]), [H * HB, 2], [1, HA]],
                            )
                            nc.tensor.matmul(
                                o_pair[:, ds(j * 512, 512)],
                                lhs,
                                p2[:, :, ds(j * 512, 512)],
                                start=(t == 0),
                                stop=(t == 3),
                                perf_mode=DR,
                            )

                    pend = []
                    p2 = None
                    for m in range(NT):
                        if m % 2 == 0:
                            p2 = pch.tile([P, 2, N], F8, tag="p2")
                        s_pair = ps_flow.tile([P, N], FP, tag="flow", name="spair")
                        for j in range(2):
                            lo = j * 64
                            nc.tensor.matmul(
                                s_pair[:, ds(j * 512, 512)],
                                kpt[lo : lo + 64, hp, ts(m, P)],
                                qpt[lo : lo + 64, hp, qslice],
                                start=True,
                                stop=True,
                            )
                        if (gi[0], m) in FASTEXP_DVE:
                            nc.vector.tensor_scalar(
                                out=p2[:, m % 2, :].bitcast(U8),
                                in0=s_pair, scalar1=FE_C1, scalar2=FE_C2,
                                op0=ALU.mult, op1=ALU.add,
                            )
                        else:
                            nc.scalar.activation(
                                p2[:, m % 2, :], s_pair, AF.Exp, scale=SCALE
                            )
                        if carry:
                            carry.pop(0)()
                        if gi[0] <= 1 and st["vp_rest"]:
                            st["vp_rest"].pop(0)()
                        if m == 2 and pending_drain[0] is not None:
                            pending_drain[0]()
                            pending_drain[0] = None
                        if hf == 0 and m == 4:
                            qp_chunk(b, hp)
                        if m % 2 == 1:
                            pend.append((m // 2, p2))
                            if len(pend) > 3:
                                emit_pv(*pend.pop(0))
                    for t, pp in pend:
                        carry.append(lambda t=t, pp=pp, f=emit_pv: f(t, pp))
                    pend = []

                    # drain: PSUM -> SBUF bf16, transpose to natural,
                    # normalize rows by 1/denominator, add the Qp residual.
                    # Deferred into the next group.
                    def make_drain(o_pair=o_pair, hp=hp, hf=hf):
                        def drain():
                            o_sb = ost.tile([HA, N], BF, tag="ost", name="osb")
                            nc.vector.tensor_copy(o_sb, o_pair)
                            t_ps = ps_acc.tile(
                                [P, 2 * DT, HB], BF, tag="acc", name="otr"
                            )
                            for blk in range(2 * DT):
                                nc.tensor.transpose(
                                    t_ps[:, blk, 0:HA],
                                    o_sb[:, ts(blk, P)],
                                    ident_b[0:HA, 0:HA],
                                )
                            r8 = sml.tile([P, 2 * DT], FP, tag="r8", name="r8")
                            den = bass.AP(
                                tensor=t_ps.tensor, offset=t_ps.offset + HD,
                                ap=[list(t_ps.ap[0]), [HB, 2 * DT]],
                            )
                            nc.vector.reciprocal(r8, den)
                            tmp = otile.tile(
                                [P, 2 * DT, HD], BF, tag="dtmp", name="dtmp"
                            )
                            nc.vector.tensor_tensor(
                                tmp,
                                bass.AP(
                                    tensor=t_ps.tensor, offset=t_ps.offset,
                                    ap=[list(t_ps.ap[0]), [HB, 2 * DT], [1, HD]],
                                ),
                                _free_bcast(r8[:], HD),
                                ALU.mult,
                            )
                            # blk = j*4+qq -> head 2hp+j, q tile hf*4+qq
                            def hcols(tl):
                                return bass.AP(
                                    tensor=tl.tensor,
                                    offset=tl.offset
                                    + (hf * 4) * D
                                    + (2 * hp) * HD,
                                    ap=[list(tl.ap[0]), [HD, 2], [D, DT], [1, HD]],
                                )
                            nc.vector.tensor_tensor(
                                hcols(oasm),
                                tmp[:].rearrange("p (j q) e -> p j q e", j=2),
                                hcols(qp),
                                ALU.add,
                            )
                        return drain

                    pending_drain[0] = make_drain()
                    if extras is not None and gi[0] in extras:
                        extras[gi[0]]()
                    gi[0] += 1

            for f in carry:
                f()
            carry = []
            if pending_drain[0] is not None:
                pending_drain[0]()
                pending_drain[0] = None
            st.update(oasm=oasm)

        def phase_c_half(b, half):
            """LN1 + FFN + LN2 + out-DMA for q tiles [4*half, 4*half+4)."""
            st = state[b]
            oasm = st["oasm"]
            QH = 4
            q0 = QH * half
            ln1 = work.tile([P, QH, D], BF, tag=f"ln1_{half}", name="ln1")
            mva = sml.tile([P, QH, 2], FP, tag=f"mva{half}", name="mva")
            for qq in range(QH):
                st_ = sml.tile([P, 6], FP, tag="bn", name="st")
                nc.vector.bn_stats(st_, oasm[:, q0 + qq, :])
                nc.vector.bn_aggr(mva[:, qq, :], st_)
            veps = sml.tile([P, QH], FP, tag=f"veps{half}", name="veps")
            nc.vector.tensor_scalar(
                out=veps, in0=mva[:, :, 1], scalar1=EPS, scalar2=None, op0=ALU.add
            )
            rsa = sml.tile([P, QH], FP, tag=f"rsa{half}", name="rsa")
            rsqrt_dve(rsa[:], veps[:], QH)
            for qq in range(QH):
                lq = ln1[:, qq, :]
                nc.vector.tensor_scalar(
                    out=lq,
                    in0=oasm[:, q0 + qq, :],
                    scalar1=mva[:, qq, 0:1],
                    scalar2=rsa[:, qq : qq + 1],
                    op0=ALU.subtract,
                    op1=ALU.mult,
                )
                if not triv0:
                    nc.vector.tensor_tensor(lq, lq, bc["g0"], ALU.mult)
                    nc.vector.tensor_tensor(lq, lq, bc["b0"], ALU.add)
            o2a = work.tile([P, QH, D], BF, tag=f"o2a{half}", name="o2a")
            mv2a = sml.tile([P, QH, 2], FP, tag=f"mv2a{half}", name="mv2a")
            for qq in range(QH):
                lq = ln1[:, qq, :]
                tp = ps_flow.tile([P, 512], BF, tag="flow", name="lntr")
                for c in range(DT):
                    nc.tensor.transpose(tp[:, ts(c, P)], lq[:, ts(c, P)], ident_b)
                l_t = lnt.tile([P, DT, P], BF, tag="lnt", name="lt")
                nc.vector.tensor_copy(l_t, tp[:].rearrange("p (c n) -> p c n", n=P))

                f_ps = ps_acc.tile([P, 512], FP, tag="acc", name="ffps")
                for c in range(DT):
                    nc.tensor.matmul(
                        f_ps,
                        l_t[:, c, :],
                        wsb["Wo"][:, c, :],
                        start=(c == 0),
                        stop=(c == DT - 1),
                    )
                o2 = o2a[:, qq, :]
                if trivbo:
                    nc.vector.scalar_tensor_tensor(
                        out=o2, in0=f_ps, scalar=0.0, in1=lq,
                        op0=ALU.max, op1=ALU.add,
                    )
                else:
                    rf = otile.tile([P, D], FP, tag="rf", name="rf")
                    nc.vector.tensor_tensor(rf, f_ps, bc["bo"], ALU.add)
                    nc.vector.scalar_tensor_tensor(
                        out=o2, in0=rf, scalar=0.0, in1=lq,
                        op0=ALU.max, op1=ALU.add,
                    )
                st2 = sml.tile([P, 6], FP, tag="bn", name="st2")
                nc.vector.bn_stats(st2, o2)
                nc.vector.bn_aggr(mv2a[:, qq, :], st2)
            ve2 = sml.tile([P, QH], FP, tag=f"ve2{half}", name="ve2")
            nc.vector.tensor_scalar(
                out=ve2, in0=mv2a[:, :, 1], scalar1=EPS, scalar2=None, op0=ALU.add
            )
            rs2 = sml.tile([P, QH], FP, tag=f"rs2{half}", name="rs2")
            rsqrt_dve(rs2[:], ve2[:], QH)
            for qq in range(QH):
                z2 = otile.tile([P, D], FP, tag="z", name="z2")
                nc.vector.tensor_scalar(
                    out=z2,
                    in0=o2a[:, qq, :],
                    scalar1=mv2a[:, qq, 0:1],
                    scalar2=rs2[:, qq : qq + 1],
                    op0=ALU.subtract,
                    op1=ALU.mult,
                )
                if not triv1:
                    nc.vector.tensor_tensor(z2, z2, bc["g1"], ALU.mult)
                    nc.vector.tensor_tensor(z2, z2, bc["b1"], ALU.add)
                nc.sync.dma_start(out=out_O[b, ts(q0 + qq, P), :], in_=z2)

        # batch 0 merges projections into phase B (compute starts with the
        # first DMA'd chunk); batch 1's projections are emitted right after
        # so the PE fills phase B0's ACT-bound stretch; batch 0's LN/FFN
        # halves then hide under batch 1's ACT-bound phase B. Batch 1 runs
        # hf-major so its first four q tiles finish mid-phase and the first
        # LN/FFN half overlaps the remaining groups.
        hp_major = [(hp, hf) for hp in range(PAIRS) for hf in range(2)]
        hf_major = [(hp, hf) for hf in range(2) for hp in range(PAIRS)]
        phase_a_load(0)
        phase_a(0, merge=True)
        phase_b(0, hp_major, extras={1: lambda: phase_a_load(1)})
        phase_a(1, merge=False)
        phase_b(
            1,
            hf_major,
            extras={
                0: lambda: phase_c_half(0, 0),
                2: lambda: phase_c_half(0, 1),
                4: lambda: phase_c_half(1, 0),
            },
        )
        phase_c_half(1, 1)

    nc.compile()
    return nc


_NC = {}


def _get_nc(triv0, triv1, trivbo):
    key = (triv0, triv1, trivbo)
    if key not in _NC:
        _NC[key] = _build_program(*key)
    return _NC[key]


def _prep_in_maps(inputs):
    import ml_dtypes

    f32 = lambda x: np.ascontiguousarray(np.asarray(x), dtype=np.float32)
    bf = lambda x: np.ascontiguousarray(
        np.asarray(x, dtype=np.float32).astype(ml_dtypes.bfloat16)
    )
    f8 = lambda x: np.ascontiguousarray(
        np.asarray(x, dtype=np.float32).astype(ml_dtypes.float8_e4m3)
    )
    Q, K = f32(inputs["Q"]), f32(inputs["K"])
    QT = np.ascontiguousarray(Q.transpose(0, 2, 1))
    KT = np.ascontiguousarray(K.transpose(0, 2, 1))
    shared = {
        "Wq": bf(inputs["Wq"]),
        "Wk": f8(inputs["Wk"]),
        "Wv": bf(inputs["Wv"]),
        "Wo": bf(inputs["Wo"]),
        "bq2": np.ascontiguousarray(f32(inputs["bq"]).reshape(DT, P).T),
        "bk2": np.ascontiguousarray(f32(inputs["bk"]).reshape(DT, P).T),
        "bv4": f32(inputs["bv"]) * 4.0,
        "bo": f32(inputs["bo"]),
        "g0": f32(inputs["g0"]),
        "b0": f32(inputs["b0"]),
        "g1": f32(inputs["g1"]),
        "b1": f32(inputs["b1"]),
    }
    in_maps = []
    for c in range(NCORES):
        m = dict(shared)
        m["QT"] = np.ascontiguousarray(
            QT[c * BL : (c + 1) * BL].astype(ml_dtypes.bfloat16)
        )
        m["KT"] = np.ascontiguousarray(
            KT[c * BL : (c + 1) * BL].astype(ml_dtypes.float8_e4m3)
        )
        in_maps.append(m)
    return in_maps


def _run(inputs, trace=False):
    triv0 = bool(
        np.all(np.asarray(inputs["g0"]) == 1.0)
        and np.all(np.asarray(inputs["b0"]) == 0.0)
    )
    triv1 = bool(
        np.all(np.asarray(inputs["g1"]) == 1.0)
        and np.all(np.asarray(inputs["b1"]) == 0.0)
    )
    trivbo = bool(np.all(np.asarray(inputs["bo"]) == 0.0))
    nc = _get_nc(triv0, triv1, trivbo)
    in_maps = _prep_in_maps(inputs)
    return run_bass_kernel_spmd(nc, in_maps, list(range(NCORES)), trace=trace)


def kernel(**inputs):
    res = _run(inputs, trace=False)
    return np.concatenate([res.results[c]["O"] for c in range(NCORES)], axis=0)
